# revision 51
# baseline (speedup 1.0000x reference)
"""Trainium2 Bass kernel for nn_Attention (batch=8, seq=1024, dim=1024, 16 heads x 64).

Strategy: pure data parallelism - one batch element per NeuronCore (8 cores),
full weights replicated, zero collectives. Per core:
  LayerNorm (f32 stats) -> qkv matmul in fp32r -> per-head RMS norm (square/
  reduce on Pool, ln/exp on ACT) -> q/k cast to bf16 and transposed to [c, t]
  layout via DMA XBAR transposes -> scores^T = k^T-stationary matmul into
  PSUM -> exp on ScalarE (no max subtraction: |s| <= 64 < 88) -> attn@v
  FLIPPED: stationary = exp(scores^T) [j, i-block], moving = v|ones [j, 65],
  so output lands [i, c|denom] with the softmax denominator per-partition ->
  one DVE divide per (head, i-block) -> pair-stacked [i, 2hx64c] tiles are
  DMA-XBAR-transposed into ohn [2hx64c, i] -> out-proj in bf16, split into a
  pairs-0..3 partial (PE filler during the ACT-bound second attention chunk,
  staged through DRAM scratch) and a pairs-4..7 tail merged with a DVE add.
All matmul accumulation is fp32 in PSUM. PE transposes use a bf16 identity
(1.0 cycles/row on the moving side, numerically exact). The only ScalarE
table set is natural_log_exp (exp/ln/square/copy). All DMAs ride HWDGE via
the SP queue, keeping the Pool engine free for square/reduce/copy offload.
"""
import sys

sys.path.insert(0, '/opt/trn_rl_repo')

import numpy as np
import ml_dtypes
import concourse.bass as bass
import concourse.mybir as mybir
import concourse.tile as tile
from concourse import bacc
from concourse.bass_utils import run_bass_kernel_spmd

f32 = mybir.dt.float32
f32r = mybir.dt.float32r
bf16 = mybir.dt.bfloat16
AX = mybir.AxisListType
ALU = mybir.AluOpType
ACTF = mybir.ActivationFunctionType

N = 1024          # tokens per core
D = 1024          # model dim
H = 16            # heads
C = 64            # head dim
NT = N // 128     # token tiles
DT = D // 128     # dim tiles

LN_EPS = 1e-5
RMS_EPS = 1e-24


def build():
    nc = bacc.Bacc(None)
    x = nc.declare_dram_parameter("x", [N, D], bf16, isOutput=False)
    wqkv = nc.declare_dram_parameter("wqkv", [D, 3 * D], f32r, isOutput=False)
    wout = nc.declare_dram_parameter("wout", [D, D], bf16, isOutput=False)
    gk = nc.declare_dram_parameter("gk", [128, 2, 512], f32, isOutput=False)
    ident = nc.declare_dram_parameter("ident", [128, 128], f32r, isOutput=False)
    identb = nc.declare_dram_parameter("identb", [128, 128], bf16, isOutput=False)
    out = nc.declare_dram_parameter("out", [N, D], bf16, isOutput=True)
    oacc_dram = nc.dram_tensor("oacc_scratch", [16 * 128, 512], bf16)

    with tile.TileContext(nc) as tc:
        with tc.tile_pool(name="persist", bufs=1) as pp, \
             tc.tile_pool(name="xin", bufs=2) as sta, \
             tc.tile_pool(name="wstream", bufs=6) as wsp, \
             tc.tile_pool(name="wo", bufs=4) as wop, \
             tc.tile_pool(name="stage", bufs=2) as stg, \
             tc.tile_pool(name="small", bufs=4) as smp, \
             tc.tile_pool(name="pts", bufs=12) as ptsp, \
             tc.tile_pool(name="stack", bufs=4) as stkp, \
             tc.tile_pool(name="osb", bufs=6) as osp, \
             tc.tile_pool(name="ob", bufs=5) as obp, \
             tc.tile_pool(name="pscore", bufs=2, space="PSUM") as pscore, \
             tc.tile_pool(name="pflex", bufs=4, space="PSUM") as pflex:

            # ---- constants / persistent tensors ----
            id_sb = pp.tile([128, 128], f32r, tag="ident")
            nc.sync.dma_start(id_sb[:], ident[:])
            idb_sb = pp.tile([128, 128], bf16, tag="identb")
            nc.sync.dma_start(idb_sb[:], identb[:])
            gk_sb = pp.tile([128, 2, 512], f32, tag="gk")
            nc.sync.dma_start(gk_sb[:], gk[:])
            eps_ln = pp.tile([128, 1], f32, tag="epsln")
            nc.gpsimd.memset(eps_ln[:], LN_EPS)
            eps_rms = pp.tile([128, 1], f32, tag="epsrms")
            nc.gpsimd.memset(eps_rms[:], RMS_EPS)
            one_c = pp.tile([128, 1], f32, tag="onec")
            nc.gpsimd.memset(one_c[:], 1.0)

            xnT = pp.tile([128, DT, N], f32r, tag="xnT")          # [d, dt, t]
            qnT = [pp.tile([128, NT, 4, 128], bf16, tag=f"qnT{c}", name=f"qnT_{c}")
                   for c in range(2)]                             # [2hx64c, tt, pair%4, t]
            knT = [pp.tile([128, NT, 4, 128], bf16, tag=f"knT{c}", name=f"knT_{c}")
                   for c in range(2)]
            v_aug = pp.tile([128, NT, H, 66], bf16, tag="vaug")   # [j, jt, h, c|1|pad]
            ohn = pp.tile([128, 8, 2, 4, 128], bf16, tag="ohn")   # [2hx64c, pair, q, b, i]

            # ---- input DMAs on the SP/HWDGE queue ----
            x_tiles = [None] * NT
            w_tiles = {}

            def load_x(tt):
                x_sb = sta.tile([128, D], bf16, tag="x_t", name=f"x_{tt}")
                nc.sync.dma_start(x_sb[:], x[tt * 128:(tt + 1) * 128, :])
                x_tiles[tt] = x_sb

            def load_w(grp):
                tiles = []
                for quarter in range(4):
                    w_sb = wsp.tile([128, DT // 4, 512], f32r, tag="wg",
                                    name=f"w_{grp}_{quarter}")
                    nc.sync.dma_start(
                        w_sb[:], wqkv[quarter * 256:(quarter + 1) * 256,
                                      grp * 512:(grp + 1) * 512]
                        .rearrange("(ko ki) f -> ki ko f", ki=128))
                    tiles.append(w_sb)
                w_tiles[grp] = tiles

            load_x(0)
            load_w(4)
            load_x(1)
            load_x(2)
            for tt in range(3, NT):
                load_x(tt)
            load_w(0)

            # ---------- LayerNorm for one token tile ----------
            def layer_norm(tt):
                ts = slice(tt * 128, (tt + 1) * 128)
                x_sb = x_tiles[tt]
                s1 = smp.tile([128, 1], f32, tag="s1")
                nc.vector.tensor_reduce(s1[:], x_sb[:], AX.X, ALU.add)
                xn_t = stg.tile([128, D], f32r, tag="xn_t")
                s2 = smp.tile([128, 1], f32, tag="s2")
                # Square scratch written into xn_t; normalize overwrites it
                nc.scalar.activation(xn_t[:], x_sb[:], ACTF.Square,
                                     bias=0.0, scale=1.0, accum_out=s2[:])
                m2 = smp.tile([128, 1], f32, tag="m2")
                nc.vector.tensor_tensor(m2[:], s1[:], s1[:], ALU.mult)
                dvar = smp.tile([128, 1], f32, tag="dvar")
                nc.vector.tensor_scalar(dvar[:], m2[:], -1.0 / D, s2[:], ALU.mult, ALU.add)
                lnv = smp.tile([128, 1], f32, tag="lnv")
                nc.scalar.activation(lnv[:], dvar[:], ACTF.Ln, bias=eps_ln[:], scale=1.0 / D)
                rsig = smp.tile([128, 1], f32, tag="rsig")
                nc.scalar.activation(rsig[:], lnv[:], ACTF.Exp, bias=0.0, scale=-0.5)
                nmr = smp.tile([128, 1], f32, tag="nmr")
                nc.vector.tensor_scalar(nmr[:], s1[:], rsig[:], -1.0 / D, ALU.mult, ALU.mult)
                nc.vector.tensor_scalar(xn_t[:, 0:512], x_sb[:, 0:512], rsig[:], nmr[:],
                                        ALU.mult, ALU.add)
                nc.scalar.activation(xn_t[:, 512:D], x_sb[:, 512:D], ACTF.Identity,
                                     bias=nmr[:], scale=rsig[:])
                return xn_t

            def xn_transposes(tt, xn_t):
                ts = slice(tt * 128, (tt + 1) * 128)
                for dt_i in range(DT):
                    ps_t = pflex.tile([128, 512], f32r, tag="pflex", name=f"pst_{tt}_{dt_i}")
                    nc.tensor.transpose(ps_t[:, 0:128], xn_t[:, dt_i * 128:(dt_i + 1) * 128],
                                        id_sb[:])
                    if dt_i % 2 == 0:
                        nc.vector.tensor_copy(xnT[:, dt_i, ts], ps_t[:, 0:128])
                    else:
                        nc.scalar.copy(xnT[:, dt_i, ts], ps_t[:, 0:128])

            # ---------- qkv projection for one (group, token-tile) ----------
            # grp: 0,1=q 2,3=k 4,5=v
            def qkv_tile(grp, tt, chunk):
                ts = slice(tt * 128, (tt + 1) * 128)
                kind = grp // 2  # 0=q, 1=k, 2=v
                w_halves = w_tiles[grp]
                ps_q = pflex.tile([128, 512], f32, tag="pflex", name=f"psq_{grp}_{tt}")
                for dt_i in range(DT):
                    nc.tensor.matmul(ps_q[:], xnT[:, dt_i, ts],
                                     w_halves[dt_i // 2][:, dt_i % 2, :],
                                     start=(dt_i == 0), stop=(dt_i == DT - 1))
                if kind == 2:
                    hbase = (grp - 4) * 8
                    # ACT during the qkv phase, DVE when run as a round filler
                    # (ACT paces attention rounds)
                    eng = nc.scalar.copy if chunk == 0 else nc.vector.tensor_copy
                    eng(v_aug[:, tt, hbase:hbase + 8, 0:64],
                        ps_q.rearrange("p (h c) -> p h c", c=64))
                    return
                # RMS stats. chunk 0 (qkv phase): square on ACT (idle
                # there; DVE is the pacer) straight from PSUM. chunk 1
                # (attention-round fillers, ACT-paced): stage ps_q into SBUF
                # first - the HW DVE cannot read two PSUM operands - then
                # square on DVE. Reduce on DVE either way.
                sq = stg.tile([128, 512], bf16, tag="sq", name=f"sq_{grp}_{tt}",
                              bufs=2)
                ss = smp.tile([128, 8], f32, tag="ss")
                if chunk == 0:
                    nc.scalar.activation(sq[:], ps_q[:], ACTF.Square,
                                         bias=0.0, scale=1.0)
                    qsrc = ps_q
                else:
                    qs = stg.tile([128, 512], bf16, tag="qs", name=f"qs_{grp}_{tt}",
                                  bufs=2)
                    nc.vector.tensor_copy(qs[:], ps_q[:])
                    nc.vector.tensor_tensor(sq[:], qs[:], qs[:], ALU.mult)
                    qsrc = qs
                nc.vector.tensor_reduce(
                    ss[:], sq.rearrange("p (h c) -> p h c", c=64), AX.X, ALU.add)
                lnss = smp.tile([128, 8], f32, tag="lnss")
                nc.scalar.activation(lnss[:], ss[:], ACTF.Ln, bias=eps_rms[:], scale=1.0)
                rsq = smp.tile([128, 8], f32, tag="rsq")
                nc.scalar.activation(rsq[:], lnss[:], ACTF.Exp, bias=0.0, scale=-0.5)
                qn_t = stg.tile([128, 512], bf16, tag="qn_t", name=f"qn_{grp}_{tt}",
                                bufs=5)
                if kind == 0:
                    nc.vector.tensor_tensor(
                        qn_t.rearrange("p (h c) -> p h c", c=64),
                        qsrc.rearrange("p (h c) -> p h c", c=64),
                        rsq[:, :, None].to_broadcast((128, 8, 64)), ALU.mult)
                    dstT = qnT[chunk]
                else:
                    # k side: normalize then fold 64*gamma_q*gamma_k
                    kn_f = stg.tile([128, 512], f32, tag="kn_f", name=f"knf_{grp}_{tt}",
                                    bufs=1)
                    nc.vector.tensor_tensor(
                        kn_f.rearrange("p (h c) -> p h c", c=64),
                        qsrc.rearrange("p (h c) -> p h c", c=64),
                        rsq[:, :, None].to_broadcast((128, 8, 64)), ALU.mult)
                    nc.vector.tensor_tensor(qn_t[:], kn_f[:], gk_sb[:, chunk, :], ALU.mult)
                    dstT = knT[chunk]
                # PE transposes (bf16 identity) + one batched copy, returned
                # as a closure so the caller can emit them ~2 units later -
                # emitting them inline would head-of-line block the in-order
                # PE stream on the multi-engine RMS chain (~3 us)
                def part2():
                    ps_t4 = pflex.tile([128, 4, 128], bf16, tag="pflex",
                                       name=f"pstq_{grp}_{tt}")
                    for blk in range(4):
                        nc.tensor.transpose(ps_t4[:, blk, :],
                                            qn_t[:, blk * 128:(blk + 1) * 128],
                                            idb_sb[:])
                    if chunk == 0:
                        nc.scalar.copy(dstT[:, tt, :, :], ps_t4[:])
                    else:
                        nc.vector.tensor_copy(dstT[:, tt, :, :], ps_t4[:])
                return part2

            # ---------- attention ----------
            def scores_jt(state, jt, chunk):
                h, p, hs, pts = state
                hp = slice(hs * 64, (hs + 1) * 64)
                pc = p % 4
                ps_s = pscore.tile([128, 1024], f32, tag="ps_s", name=f"pss_{h}_{jt}")
                for ih in range(2):
                    nc.tensor.matmul(ps_s[:, ih * 512:(ih + 1) * 512],
                                     knT[chunk][hp, jt, pc, :],
                                     qnT[chunk][hp, ih * 4:(ih + 1) * 4, pc, :],
                                     start=True, stop=True)
                nc.scalar.activation(pts[jt][:], ps_s[:], ACTF.Exp,
                                     bias=0.0, scale=1.0)

            stacked = {}
            flush_ps_o = []

            def attnv_jts(state, jt):
                """One jt-slice of the final head's attn@v, interleaved into
                its own round (used only for the flush)."""
                h, p, hs, pts = state
                if jt == 0:
                    for half in range(2):
                        flush_ps_o.append(
                            pflex.tile([128, 4, 66], f32, tag="pflex",
                                       name=f"pso_{h}_{half}"))
                for it in range(NT):
                    nc.tensor.matmul(
                        flush_ps_o[it // 4][:, it % 4, :],
                        pts[jt][:, it * 128:(it + 1) * 128],
                        v_aug[:, jt, h, 0:66],
                        start=(jt == 0 and it % 4 == 0),
                        stop=(jt == NT - 1 and it % 4 == 3),
                        skip_group_check=True)

            def attnv_flush_finish(state):
                h, p, hs, pts = state
                for jt in range(NT - 3, NT):
                    attnv_jts(state, jt)
                ps_o = flush_ps_o
                recips = []
                for q in range(2):
                    rc = smp.tile([128, 4], f32, tag="recip", name=f"rc_{h}_{q}")
                    nc.vector.reciprocal_approx_fast(
                        rc[:], ps_o[q][:, :, 64:65].rearrange("p a b -> p (a b)"))
                    recips.append(rc)
                for it in range(NT):
                    q = it // 4
                    if (p, q) not in stacked:
                        stacked[(p, q)] = stkp.tile([128, 4, 128], bf16, tag="stk",
                                                    name=f"stk_{p}_{q}")
                    stk = stacked[(p, q)]
                    nc.vector.tensor_scalar(
                        stk[:, it % 4, hs * 64:(hs + 1) * 64],
                        ps_o[q][:, it % 4, 0:64],
                        recips[q][:, it % 4:it % 4 + 1],
                        None, ALU.mult)
                for q in range(2):
                    stk = stacked[(p, q)]
                    ps_t4 = pflex.tile([128, 4, 128], bf16, tag="pflex",
                                       name=f"psto_{p}_{q}")
                    for b in range(4):
                        nc.tensor.transpose(ps_t4[:, b, :], stk[:, b, :],
                                            idb_sb[:])
                    nc.vector.tensor_copy(ohn[:, p, q, :, :], ps_t4[:])
                    del stacked[(p, q)]

            def attnv_burst(state):
                """attn@v for one whole head as a compact burst: 64 small
                matmuls (it-outer) into two [128, 4, 66] PSUM tiles, then the
                batched reciprocal + 8 normalize-mults into the pair-stacked
                bf16 tiles, then (for the second head of a pair) the PE
                transposes of completed quads into ohn. Keeping the burst
                compact keeps every PSUM tile short-lived so the shared
                4-slot pool never stalls the in-order PE stream."""
                h, p, hs, pts = state
                ps_o = [pflex.tile([128, 4, 66], f32, tag="pflex",
                                   name=f"pso_{h}_{half}") for half in range(2)]
                # start marks the whole 2KB PSUM bank pending-zero, so only
                # the first region of each bank may set it; stop only on the
                # very last matmul touching the bank
                for it in range(NT):
                    for jt in range(NT):
                        nc.tensor.matmul(
                            ps_o[it // 4][:, it % 4, :],
                            pts[jt][:, it * 128:(it + 1) * 128],
                            v_aug[:, jt, h, 0:66],
                            start=(it % 4 == 0 and jt == 0),
                            stop=(it % 4 == 3 and jt == NT - 1),
                            skip_group_check=True)
                recips = []
                for q in range(2):
                    rc = smp.tile([128, 4], f32, tag="recip", name=f"rc_{h}_{q}")
                    nc.vector.reciprocal_approx_fast(
                        rc[:], ps_o[q][:, :, 64:65].rearrange("p a b -> p (a b)"))
                    recips.append(rc)
                for it in range(NT):
                    q = it // 4
                    if (p, q) not in stacked:
                        stacked[(p, q)] = stkp.tile([128, 4, 128], bf16, tag="stk",
                                                    name=f"stk_{p}_{q}")
                    stk = stacked[(p, q)]
                    nc.vector.tensor_scalar(
                        stk[:, it % 4, hs * 64:(hs + 1) * 64],
                        ps_o[q][:, it % 4, 0:64],
                        recips[q][:, it % 4:it % 4 + 1],
                        None, ALU.mult)
                if hs != 1:
                    return None

                def do_psto():
                    for q in range(2):
                        stk = stacked[(p, q)]
                        ps_t4 = pflex.tile([128, 4, 128], bf16, tag="pflex",
                                           name=f"psto_{p}_{q}")
                        for b in range(4):
                            nc.tensor.transpose(ps_t4[:, b, :], stk[:, b, :],
                                                idb_sb[:])
                        nc.vector.tensor_copy(ohn[:, p, q, :, :], ps_t4[:])
                        del stacked[(p, q)]
                return do_psto

            # ---------- output projection ----------
            wout_q = []

            def load_wout():
                for q in range(4):
                    w_sb = wop.tile([128, 2, D], bf16, tag="wo", name=f"wo_{q}")
                    nc.sync.dma_start(
                        w_sb[:], wout[q * 256:(q + 1) * 256, :]
                        .rearrange("(ko ki) d -> ki ko d", ki=128))
                    wout_q.append(w_sb)

            def proj_a(u):
                """Accumulate pairs 0..3 for (it, dh) unit u into DRAM scratch."""
                it, dh = u // 2, u % 2
                its = slice(it * 128, (it + 1) * 128)
                ps_h = pflex.tile([128, 512], f32, tag="pflex", name=f"psh_{u}")
                for p in range(4):
                    nc.tensor.matmul(
                        ps_h[:], ohn[:, p, it // 4, it % 4, :],
                        wout_q[p // 2][:, p % 2, dh * 512:(dh + 1) * 512],
                        start=(p == 0), stop=(p == 3))
                oa = obp.tile([128, 512], bf16, tag="ob", name=f"oa_{u}")
                nc.vector.tensor_copy(oa[:], ps_h[:])
                nc.sync.dma_start(oacc_dram[u * 128:(u + 1) * 128, :], oa[:])

            ob_tiles = {}

            def ob_load(u):
                ob = obp.tile([128, 512], bf16, tag="ob", name=f"ob_{u}")
                nc.sync.dma_start(ob[:], oacc_dram[u * 128:(u + 1) * 128, :])
                ob_tiles[u] = ob

            def proj_b(u):
                it, dh = u // 2, u % 2
                its = slice(it * 128, (it + 1) * 128)
                ob = ob_tiles.pop(u)
                ps_f = pflex.tile([128, 512], f32, tag="pflex", name=f"psf_{u}")
                for p in range(4, 8):
                    nc.tensor.matmul(
                        ps_f[:], ohn[:, p, it // 4, it % 4, :],
                        wout_q[p // 2][:, p % 2, dh * 512:(dh + 1) * 512],
                        start=(p == 4), stop=(p == 7))
                o_sb = osp.tile([128, 512], bf16, tag="o_sb", name=f"osb_{u}")
                nc.vector.tensor_tensor(o_sb[:], ps_f[:], ob[:], ALU.add)
                nc.sync.dma_start(out[its, dh * 512:(dh + 1) * 512], o_sb[:])

            # ================= schedule =================
            from collections import deque
            pending2 = deque()

            def drain2(keep):
                while len(pending2) > keep:
                    pending2.popleft()()

            # Phase A: LN stats pipelined one tile ahead of transposes + v1
            xn_prev = None
            for tt in range(NT):
                xn_t = layer_norm(tt)
                if xn_prev is not None:
                    xn_transposes(tt - 1, xn_prev)
                    qkv_tile(4, tt - 1, 0)
                xn_prev = xn_t
            xn_transposes(NT - 1, xn_prev)
            qkv_tile(4, NT - 1, 0)
            nc.gpsimd.tensor_copy(
                v_aug[:, :, :, 64:65].rearrange("p a b o -> p (a b o)"),
                one_c[:].to_broadcast((128, NT * H)))
            nc.gpsimd.memset(
                v_aug[:, :, :, 65:66].rearrange("p a b o -> p (a b o)"), 0.0)
            # q/k for heads 0..7, v for heads 8..15; transposes lag 2 units,
            # weight groups load as earlier groups' pool slots free up
            for tt in range(NT):
                pending2.append(qkv_tile(0, tt, 0))
                drain2(3)
                if tt == 1:
                    load_w(2)
            for tt in range(NT):
                pending2.append(qkv_tile(2, tt, 0))
                drain2(3)
                if tt == 1:
                    load_w(5)
            for tt in range(NT - 4):
                qkv_tile(5, tt, 0)
                drain2(0 if tt > 1 else 2)
                if tt == 1:
                    load_w(1)
            load_wout()

            # Interleaved attention rounds. One round = scores+exp for head h
            # at jt granularity, with attn@v of the previous head and PE
            # filler units spread between the score matmuls so PE never
            # head-of-line blocks on the 2-slot score PSUM pool.
            prev = None

            def attention_round(p, hs, chunk, fillers, final=False):
                nonlocal prev
                h = 2 * p + hs
                state = (h, p, hs,
                         [ptsp.tile([128, N], bf16, tag="pts",
                                    name=f"pts_{h}_{jt}") for jt in range(NT)])
                fill = list(fillers)
                psto = None
                for jt in range(NT):
                    scores_jt(state, jt, chunk)
                    if jt == 2 and prev is not None:
                        psto = attnv_burst(prev)
                    if jt == 5 and psto is not None:
                        # quad transposes deferred so PE is not stalled on the
                        # divide burst right after the attn@v matmuls
                        psto()
                        psto = None
                    if final and jt >= 3:
                        # self-interleave the last head: its attn@v lags its
                        # own exps by 2 score tiles, so no separate flush
                        # burst has to wait for the final exp
                        attnv_jts(state, jt - 3)
                    if jt in (3, 4, 5, 6) and fill:
                        fill.pop(0)()
                for f in fill:
                    f()
                if psto is not None:
                    psto()
                prev = state

            # chunk 1: pairs 0..3; filler = qkv groups 1 and 3 (chunk 2 q/k),
            # with each unit's transposes lagging one round behind its part1.
            # w3 is loaded between fillers, right as w1's pool slots free up
            # (issuing it earlier would block the in-order SP DMA queue).
            qkv_fill = ([(5, tt) for tt in range(NT - 4, NT)]
                        + [(1, tt) for tt in range(NT)] + [(3, tt) for tt in range(NT)])
            fi = 0

            def filler_p1(g, t):
                p2 = qkv_tile(g, t, 1)
                if p2 is not None:
                    pending2.append(p2)

            def filler_p2():
                # keep 3 in flight so a unit's transposes run well after its
                # part1 (the RMS chain must clear first)
                if len(pending2) > 3:
                    pending2.popleft()()

            for r in range(8):
                p, hs = r // 2, r % 2
                n1 = 3 if r < 4 else 2
                fillers = [
                    (lambda g=g, t=t: filler_p1(g, t))
                    for g, t in qkv_fill[fi:fi + n1]]
                fillers.append(filler_p2)
                fillers.append(filler_p2)
                fi += n1
                attention_round(p, hs, 0, fillers)
                if fi == 12:
                    load_w(3)
            drain2(0)
            # chunk 2: pairs 4..7; filler = proj part A (pairs 0..3), which
            # becomes available once chunk 1's last pair transposes land
            proj_sched = [0, 3, 3, 2, 2, 2, 2, 2]
            pu = 0
            for p in range(4, 8):
                for hs in range(2):
                    n = proj_sched[(p - 4) * 2 + hs]
                    fillers = [(lambda u=u: proj_a(u)) for u in range(pu, pu + n)]
                    pu += n
                    attention_round(p, hs, 1, fillers,
                                    final=(p == 7 and hs == 1))
            attnv_flush_finish(prev)

            # tail: part B + merge + store, with the DRAM partials
            # prefetched a few units ahead so the in-order SP queue never
            # serializes a load behind the previous unit's store
            for u in range(3):
                ob_load(u)
            for u in range(16):
                if u + 3 < 16:
                    ob_load(u + 3)
                proj_b(u)
    return nc


_NC_CACHE = None


def _patch_act_tables():
    """Steer bacc's greedy act-table-set selection to natural_log_exp_and_others
    for every function this kernel uses (exp/ln/square/copy/identity), by
    hiding those functions from all earlier sets. Set order (and thus the
    act_func_set_id each load emits) is unchanged, so the runtime tables are
    correct - but all our activations resolve to one co-resident set and the
    kernel performs a single table load instead of thrashing."""
    import collections
    import concourse.bacc as _bacc
    import concourse.hw_specs as _hw
    orig = getattr(_hw.get_activation_tables, '__wrapped_orig__', _hw.get_activation_tables)

    def patched(arch):
        d = orig(arch)
        key = "natural_log_exp_and_others"
        if key not in d:
            return d
        mine = d[key]
        hidden = {f for f in mine}
        nd = collections.OrderedDict()
        for k, v in d.items():
            if k == key:
                nd[k] = v
            else:
                nd[k] = v - hidden
        return nd
    patched.__wrapped_orig__ = orig
    _hw.get_activation_tables = patched
    _bacc.get_activation_tables = patched


def _get_nc():
    global _NC_CACHE
    if _NC_CACHE is None:
        _patch_act_tables()
        nc = build()
        nc.finalize()
        _NC_CACHE = nc
    return _NC_CACHE


def kernel(x, ln_gamma, q_gamma, k_gamma, w_qkv, w_out):
    x = np.asarray(x, dtype=np.float32)
    ln_gamma = np.asarray(ln_gamma, dtype=np.float32)
    q_gamma = np.asarray(q_gamma, dtype=np.float32).reshape(H, C)
    k_gamma = np.asarray(k_gamma, dtype=np.float32).reshape(H, C)
    w_qkv = np.asarray(w_qkv, dtype=np.float32)
    w_out = np.asarray(w_out, dtype=np.float32)

    wqkv_eff = np.ascontiguousarray(ln_gamma[:, None] * w_qkv, dtype=np.float32)
    wout_bf = w_out.astype(ml_dtypes.bfloat16)
    # 64*gamma_q*gamma_k laid out [t-partition(bcast), chunk, (8h x 64c)]
    g_flat = (64.0 * q_gamma * k_gamma).reshape(2, 512)
    gk_full = np.ascontiguousarray(
        np.broadcast_to(g_flat[None, :, :], (128, 2, 512)), dtype=np.float32)
    ident = np.eye(128, dtype=np.float32)
    identb = np.eye(128, dtype=np.float32).astype(ml_dtypes.bfloat16)

    nc = _get_nc()
    in_maps = [
        {"x": np.ascontiguousarray(x[i]).astype(ml_dtypes.bfloat16), "wqkv": wqkv_eff, "wout": wout_bf,
         "gk": gk_full, "ident": ident, "identb": identb}
        for i in range(8)
    ]
    res = run_bass_kernel_spmd(nc, in_maps, core_ids=list(range(8)))
    return np.stack([np.asarray(res.results[i]["out"]).astype(np.float32)
                     for i in range(8)], axis=0)
